# revision 9
# baseline (speedup 1.0000x reference)
"""Trainium2 Bass kernel for nn_DetectionLoss (nms_detection).

Strategy (data-parallel over batch, 8 cores x 4 images):
  - Each core builds its [3600, 1024] slab of the cost volume
    C = 1*cost_class + 5*cost_bbox + 2*cost_giou and partial loss sums.
  - The pairwise grid runs in fp16 spread across THREE engines:
      Act:  4x |coord - q| (abs w/ bias), ln/exp pair for the two
            reciprocals (2/union, 2/enclose), packed [128,2048]
      DVE:  tensor_scalar (4x mode, 327ns) + tensor_tensor (2x, 594ns)
            for the sums/min-max/products, final combine STT
      Pool: the two products i1/e1, area tensor_scalar, final row sum
  - cost_class via PE matmul of resident f16 transposed region features
    against normalized projected text; row-norm division and the "+2"
    giou constant are folded into the final fused combine op.
  - Scalar losses are per-core partial sums combined on host; host only
    shards/transposes/concats/casts.

kernel(**inputs) takes FULL inputs (as in reference setup_inputs()) and
returns the FULL flat output [32*900*1024 + 4].
"""

import math
from contextlib import ExitStack

import numpy as np

import concourse.bass as bass
import concourse.bacc as bacc
import concourse.tile as tile
from concourse import mybir

# All activation funcs used here (Abs/Exp/Ln/Relu/Square/Identity) live in
# one table set; restricting the chooser to it avoids per-op table thrash.
_orig_gat = bacc.get_activation_tables


def _gat_single_set(arch):
    t = _orig_gat(arch)
    name = "natural_log_exp_and_others"
    if name not in t:
        return t
    return {k: (v if k == name else set()) for k, v in t.items()}


bacc.get_activation_tables = _gat_single_set
from concourse.bass_utils import run_bass_kernel_spmd
from concourse.masks import make_identity

# ---- problem constants (hardcoded; kernel.py must be self-contained) ----
B, Q, T, NNEG = 32, 900, 32, 10
RD, TD, PD = 256, 512, 256
TEMP = 0.07
NCORES = 8
BL = B // NCORES          # images per core = 4
QL = BL * Q               # queries per core = 3600
NT = (QL + 127) // 128    # 29 q-tiles per core
QP = NT * 128             # padded queries = 3712
J = B * T                 # 1024 targets (global)
ML = BL * T               # matched rows per core = 128
NL = BL * NNEG            # neg rows per core = 40
KT = TD // 128            # 4 k-chunks for text projection
CS = 5.0                  # coord scale (folds the L1 cost weight)

F32 = mybir.dt.float32
F16 = mybir.dt.float16
I32 = mybir.dt.int32
AF = mybir.ActivationFunctionType
OP = mybir.AluOpType

LN2 = float(math.log(2.0))


def build_program(loop_tiles=NT, losses=True):
    nc = bacc.Bacc("TRN2", target_bir_lowering=False, debug=False,
                   num_devices=NCORES)

    def din(name, shape, dt=F32):
        return nc.dram_tensor(name, shape, dt, kind="ExternalInput").ap()

    def dout(name, shape, dt=F32):
        return nc.dram_tensor(name, shape, dt, kind="ExternalOutput").ap()

    ins = dict(
        reg_n=din("reg_n", [QP, RD], F16),     # local region feats (padded)
        reg_t=din("reg_t", [RD, QP], F16),     # transposed local region feats
        bboxm=din("bboxm", [128, NT * 4]),     # local bbox, tile-marshalled
        bbox_rows=din("bbox_rows", [QP, 4]),   # local bbox, row layout
        clsm=din("clsm", [128, NT]),           # local cls logits, marshalled
        tgt_T=din("tgt_T", [4, J]),            # all target boxes, coord-major
        tgt_loc=din("tgt_loc", [ML, 4]),       # local target boxes row-major
        text_T=din("text_T", [TD, B]),         # all text emb, transposed
        text_rep_T=din("text_rep_T", [TD, ML]),  # local text, repeated+T
        Wt=din("Wt", [TD, PD]),
        bt_row=din("bt_row", [1, PD]),
        gidx=din("gidx", [ML, 1], I32),        # local matched row indices
        ngidx=din("ngidx", [NL, 1], I32),      # local negative row indices
    )
    outs = dict(
        C_out=dout("C_out", [QP, J], F16),
        loss_out=dout("loss_out", [64]),
        mask_scratch=dout("mask_scratch", [QP, 1]),
    )

    with tile.TileContext(nc) as tc:
        with ExitStack() as ctx:
            detection_kernel(ctx, tc, outs, ins, loop_tiles=loop_tiles,
                             losses=losses)
    nc.compile()
    return nc


def detection_kernel(ctx: ExitStack, tc: tile.TileContext, outs, ins,
                     loop_tiles=NT, losses=True):
    import os as _os
    nc = tc.nc
    singles = ctx.enter_context(tc.tile_pool(name="singles", bufs=1))
    w1 = ctx.enter_context(tc.tile_pool(
        name="w1", bufs=int(_os.environ.get("K_W1BUFS", "3"))))
    wL = ctx.enter_context(tc.tile_pool(
        name="wL", bufs=int(_os.environ.get("K_WLBUFS", "4"))))
    outp = ctx.enter_context(tc.tile_pool(
        name="outp", bufs=int(_os.environ.get("K_OUTBUFS", "2"))))
    psum = ctx.enter_context(tc.tile_pool(
        name="psum", bufs=int(_os.environ.get("K_PSBUFS", "3")), space="PSUM"))
    psum1 = ctx.enter_context(tc.tile_pool(name="psum1", bufs=1, space="PSUM"))

    ident = singles.tile([128, 128], F32)
    make_identity(nc, ident[:])

    # ---------------- broadcast target tiles (f16, coords x5) -------------
    def bcast_row(r):
        t = ins["tgt_T"]
        return bass.AP(tensor=t.tensor, offset=r * J, ap=[[0, 128], [1, J]])

    XY1 = singles.tile([128, 2 * J], F16)   # [5*x1_t | 5*y1_t]
    XY2 = singles.tile([128, 2 * J], F16)   # [5*x2_t | 5*y2_t]
    WHb = singles.tile([128, 2 * J], F16)   # [5*w_t | 5*h_t]
    H = J // 2
    stgA = singles.tile([128, H], F32, tag="stgA")
    for dst, r in ((XY1[:, 0:J], 0), (XY1[:, J:2 * J], 1),
                   (XY2[:, 0:J], 2), (XY2[:, J:2 * J], 3)):
        for c in range(2):
            t_ = ins["tgt_T"]
            src = bass.AP(tensor=t_.tensor, offset=r * J + c * H,
                          ap=[[0, 128], [1, H]])
            nc.sync.dma_start(out=stgA, in_=src)
            nc.vector.tensor_scalar(out=dst[:, c * H:(c + 1) * H], in0=stgA,
                                    scalar1=CS, scalar2=None, op0=OP.mult)
    nc.vector.tensor_sub(WHb, XY2, XY1)
    AT1b = singles.tile([128, J], F16)      # 25 * w_t * h_t
    nc.vector.tensor_mul(AT1b, WHb[:, 0:J], WHb[:, J:2 * J])

    # ---------------- query-side per-partition scalars --------------------
    bbm = singles.tile([128, NT * 4], F32)
    nc.sync.dma_start(out=bbm, in_=ins["bboxm"][:, :])
    negc5 = singles.tile([128, NT * 4], F32)
    nc.vector.tensor_scalar(out=negc5, in0=bbm, scalar1=-CS, scalar2=None,
                            op0=OP.mult)
    negc5r = negc5[:].rearrange("p (t c) -> p t c", c=4)
    wqa = singles.tile([128, NT], F32)      # 5*w_q
    hqa = singles.tile([128, NT], F32)      # 5*h_q
    aq1 = singles.tile([128, NT], F32)      # 25*area_q
    nc.vector.tensor_sub(wqa, negc5r[:, :, 0], negc5r[:, :, 2])
    nc.vector.tensor_sub(hqa, negc5r[:, :, 1], negc5r[:, :, 3])
    nc.vector.tensor_mul(aq1, wqa, hqa)

    # ---------------- resident region features (f16) ----------------------
    regT0 = singles.tile([128, QP], F16)
    regT1 = singles.tile([128, QP], F16)
    nc.sync.dma_start(out=regT0, in_=ins["reg_t"][0:128, :])
    nc.sync.dma_start(out=regT1, in_=ins["reg_t"][128:256, :])

    # ---------------- region norms (column layout) ------------------------
    n2c = singles.tile([128, NT], F32)
    junkR = singles.tile([128, RD], F32, tag="junkR")
    for t in range(NT):
        regn = w1.tile([128, RD], F16, tag="regn")
        nc.sync.dma_start(out=regn, in_=ins["reg_n"][t * 128:(t + 1) * 128, :])
        nc.scalar.activation(out=junkR, in_=regn, func=AF.Square,
                             accum_out=n2c[:, t:t + 1])
    lnn2 = singles.tile([128, NT], F32)
    nc.scalar.activation(out=lnn2, in_=n2c, func=AF.Ln)
    ninv = singles.tile([128, NT], F32)
    nc.scalar.activation(out=ninv, in_=lnn2, func=AF.Exp, scale=-0.5)
    ninvn = singles.tile([128, NT], F32)   # -1/norm
    nc.vector.tensor_scalar(out=ninvn, in0=ninv, scalar1=-1.0, scalar2=None,
                            op0=OP.mult)
    nrm2x = singles.tile([128, NT], F32)   # 2*norm
    ln2b = singles.tile([128, 1], F32)
    nc.vector.memset(ln2b, LN2)
    nc.scalar.activation(out=nrm2x, in_=lnn2, func=AF.Exp, scale=0.5,
                         bias=ln2b[:])
    # transpose -> [NT, 128] -> flatten to f16 row [1, QP]
    ps_nt = psum1.tile([NT, 128], F32, tag="ps_one")
    nc.tensor.transpose(out=ps_nt[:], in_=nrm2x[:], identity=ident[:])
    nm2T = singles.tile([NT, 128], F16)
    nc.vector.tensor_copy(out=nm2T, in_=ps_nt)
    row2n = singles.tile([1, QP], F16)
    nc.sync.dma_start(out=row2n, in_=nm2T[:])
    negones = singles.tile([1, T], F16)
    nc.vector.memset(negones, -1.0)

    # ---------------- text: all-image normalized projection ---------------
    wt_s = singles.tile([128, KT * PD], F32)
    for k in range(KT):
        nc.sync.dma_start(out=wt_s[:, k * PD:(k + 1) * PD],
                          in_=ins["Wt"][k * 128:(k + 1) * 128, :])
    bt_s = singles.tile([1, PD], F32)
    nc.sync.dma_start(out=bt_s, in_=ins["bt_row"][:, :])
    ones1 = singles.tile([1, B], F32)
    nc.vector.memset(ones1, 1.0)

    txtT_s = singles.tile([128, KT * B], F32)
    for k in range(KT):
        nc.sync.dma_start(out=txtT_s[:, k * B:(k + 1) * B],
                          in_=ins["text_T"][k * 128:(k + 1) * 128, :])
    ps_txt = psum1.tile([B, PD], F32, tag="ps_one")
    for k in range(KT):
        nc.tensor.matmul(out=ps_txt[:], lhsT=txtT_s[:, k * B:(k + 1) * B],
                         rhs=wt_s[:, k * PD:(k + 1) * PD],
                         start=(k == 0), stop=False)
    nc.tensor.matmul(out=ps_txt[:], lhsT=ones1[:], rhs=bt_s[:],
                     start=False, stop=True)
    txtp = singles.tile([B, PD], F32)
    nc.vector.tensor_copy(out=txtp, in_=ps_txt)
    junkB = junkR[0:B, :]
    n2t = singles.tile([B, 1], F32)
    nc.scalar.activation(out=junkB, in_=txtp, func=AF.Square,
                         accum_out=n2t[:])
    lnt = singles.tile([B, 1], F32)
    nc.scalar.activation(out=lnt, in_=n2t, func=AF.Ln)
    nit = singles.tile([B, 1], F32)
    nc.scalar.activation(out=nit, in_=lnt, func=AF.Exp, scale=-0.5)
    txtn = singles.tile([B, PD], F32)
    nc.vector.tensor_scalar(out=txtn, in0=txtp, scalar1=nit[:],
                            scalar2=None, op0=OP.mult)
    # transpose to [PD, B]: f16 for the grid matmuls, f32 for the loss
    txtT0 = singles.tile([128, B], F16)
    txtT1 = singles.tile([128, B], F16)
    txtT0_32 = singles.tile([128, B], F32)
    txtT1_32 = singles.tile([128, B], F32)
    for k, (dst16, dst32) in ((0, (txtT0, txtT0_32)), (1, (txtT1, txtT1_32))):
        ps_tt = psum1.tile([128, B], F32, tag="ps_one")
        nc.tensor.transpose(out=ps_tt[:], in_=txtn[:, k * 128:(k + 1) * 128],
                            identity=ident[0:B, 0:B])
        nc.vector.tensor_copy(out=dst16, in_=ps_tt)
        nc.vector.tensor_copy(out=dst32, in_=ps_tt)

    if losses:
        emit_losses(nc, tc, ctx, singles, psum1, outs, ins, wt_s, bt_s,
                    junkR, txtT0_32, txtT1_32, ident)

    # ================= main pairwise grid loop =================
    J2 = 2 * J
    for t in range(loop_tiles):
        sl = slice(t * 128, (t + 1) * 128)
        bx1 = negc5r[:, t, 0:1]
        by1 = negc5r[:, t, 1:2]
        bx2 = negc5r[:, t, 2:3]
        by2 = negc5r[:, t, 3:4]
        wq = wqa[:, t:t + 1]
        hq = hqa[:, t:t + 1]
        aq = aq1[:, t:t + 1]

        # class-cost matmul into PSUM: raw_cc - 2*norm_q
        ps_cc = psum.tile([128, T], F32, tag="ps_cc")
        nc.tensor.matmul(out=ps_cc[:], lhsT=regT0[:, sl], rhs=txtT0[:],
                         start=True, stop=False)
        nc.tensor.matmul(out=ps_cc[:], lhsT=regT1[:, sl], rhs=txtT1[:],
                         start=False, stop=False)
        nc.tensor.matmul(out=ps_cc[:], lhsT=row2n[0:1, sl], rhs=negones[:],
                         start=False, stop=True)

        # Act: 4 abs-diffs (bias = -5*coord_q), packed [dx|dy]
        D1 = w1.tile([128, J2], F16, tag="D1")
        D2 = w1.tile([128, J2], F16, tag="D2")
        nc.scalar.activation(out=D1[:, 0:J], in_=XY1[:, 0:J], func=AF.Abs,
                             bias=bx1)
        nc.scalar.activation(out=D1[:, J:J2], in_=XY1[:, J:J2], func=AF.Abs,
                             bias=by1)
        nc.scalar.activation(out=D2[:, 0:J], in_=XY2[:, 0:J], func=AF.Abs,
                             bias=bx2)
        nc.scalar.activation(out=D2[:, J:J2], in_=XY2[:, J:J2], func=AF.Abs,
                             bias=by2)

        # DVE: u = dx1+dx2 | dy1+dy2 ; SWH = [Wb+wq | Hb+hq]
        u_xy = wL.tile([128, J2], F16, tag="u_xy")
        nc.vector.tensor_add(u_xy, D1, D2)
        SWH = w1.tile([128, J2], F16, tag="SWH")
        nc.vector.tensor_scalar(out=SWH[:, 0:J], in0=WHb[:, 0:J], scalar1=wq,
                                scalar2=None, op0=OP.add)
        nc.vector.tensor_scalar(out=SWH[:, J:J2], in0=WHb[:, J:J2],
                                scalar1=hq, scalar2=None, op0=OP.add)
        WIR = w1.tile([128, J2], F16, tag="WIR")
        nc.vector.tensor_sub(WIR, SWH, u_xy)      # 2x intersect w|h (x5)
        WEN = wL.tile([128, J2], F16, tag="WEN")
        nc.vector.tensor_add(WEN, SWH, u_xy)      # 2x enclose w|h (x5)
        WH = w1.tile([128, J2], F16, tag="D1")
        nc.vector.tensor_scalar(out=WH, in0=WIR, scalar1=0.0, scalar2=None,
                                op0=OP.max)

        # Pool: products + area; W3 = [i1 | M | e1] at 100x scale
        W3 = wL.tile([128, 3 * J], F16, tag="W3")
        A1 = w1.tile([128, J], F16, tag="A1")
        nc.gpsimd.tensor_tensor(out=W3[:, 0:J], in0=WH[:, 0:J],
                                in1=WH[:, J:J2], op=OP.mult)      # i1
        nc.gpsimd.tensor_tensor(out=W3[:, 2 * J:3 * J], in0=WEN[:, 0:J],
                                in1=WEN[:, J:J2], op=OP.mult)     # e1
        nc.gpsimd.tensor_scalar(out=A1, in0=AT1b, scalar1=aq, scalar2=4.0,
                                op0=OP.add, op1=OP.mult)          # 100*(at+aq)
        # DVE: M = a1 - i1 = 100*union
        nc.vector.tensor_sub(W3[:, J:2 * J], A1, W3[:, 0:J])

        # Act: R2 = [2/M | 2/e1] via exp(ln2 - ln(x))
        LN = w1.tile([128, J2], F16, tag="LN")
        nc.scalar.activation(out=LN, in_=W3[:, J:3 * J], func=AF.Ln)
        R2 = wL.tile([128, J2], F16, tag="R2")
        nc.scalar.activation(out=R2, in_=LN, func=AF.Exp, scale=-1.0,
                             bias=ln2b[:])

        # DVE: T12 = [i1|M] * R2 = [2*iou | 2*u/e] ; pq = u_xy - T12
        T12 = w1.tile([128, J2], F16, tag="D2")
        nc.vector.tensor_mul(T12, W3[:, 0:2 * J], R2)
        pq = w1.tile([128, J2], F16, tag="SWH")
        nc.vector.tensor_sub(pq, u_xy, T12)
        # Pool: accf = pq.x + pq.y  (= l1 - 2iou - 2u/e)
        accf = w1.tile([128, J], F16, tag="A1")
        nc.gpsimd.tensor_tensor(out=accf, in0=pq[:, 0:J], in1=pq[:, J:J2],
                                op=OP.add)

        # DVE: C = ninvn*(cc - 2n) + accf = -cc/n + 2 + accf
        Cot = outp.tile([128, J], F16, tag="Cot")
        cc_b = bass.AP(tensor=ps_cc.tensor, offset=ps_cc.offset,
                       ap=[ps_cc.ap[0], [1, T], [0, T]])
        nc.vector.scalar_tensor_tensor(
            out=Cot[:].rearrange("p (a b) -> p a b", b=T),
            in0=cc_b, scalar=ninvn[:, t:t + 1],
            in1=accf[:].rearrange("p (a b) -> p a b", b=T),
            op0=OP.mult, op1=OP.add)
        nc.sync.dma_start(out=outs["C_out"][sl, :], in_=Cot[:])


def emit_losses(nc, tc, ctx, singles, psum1, outs, ins, wt_s, bt_s, junkR,
                txtT0_32, txtT1_32, ident):
    # ---------------- text: local repeated normalized projection ----------
    txtRT_s = singles.tile([128, KT * ML], F32)
    for k in range(KT):
        nc.sync.dma_start(out=txtRT_s[:, k * ML:(k + 1) * ML],
                          in_=ins["text_rep_T"][k * 128:(k + 1) * 128, :])
    onesM = singles.tile([1, ML], F32)
    nc.vector.memset(onesM, 1.0)
    ps_txr = psum1.tile([ML, PD], F32, tag="ps_one")
    for k in range(KT):
        nc.tensor.matmul(out=ps_txr[:], lhsT=txtRT_s[:, k * ML:(k + 1) * ML],
                         rhs=wt_s[:, k * PD:(k + 1) * PD],
                         start=(k == 0), stop=False)
    nc.tensor.matmul(out=ps_txr[:], lhsT=onesM[:], rhs=bt_s[:],
                     start=False, stop=True)
    txrp = singles.tile([ML, PD], F32)
    nc.vector.tensor_copy(out=txrp, in_=ps_txr)
    junkM = junkR[:, :]
    n2r = singles.tile([ML, 1], F32)
    nc.scalar.activation(out=junkM, in_=txrp, func=AF.Square,
                         accum_out=n2r[:])
    lnr = singles.tile([ML, 1], F32)
    nc.scalar.activation(out=lnr, in_=n2r, func=AF.Ln)
    nir = singles.tile([ML, 1], F32)
    nc.scalar.activation(out=nir, in_=lnr, func=AF.Exp, scale=-0.5)
    txtrep = singles.tile([ML, PD], F32)   # normalized, pre-scaled by 1/TEMP
    nc.vector.tensor_scalar(out=txtrep, in0=txrp, scalar1=nir[:],
                            scalar2=1.0 / TEMP, op0=OP.mult, op1=OP.mult)

    # ---------------- gathers: pos / neg regions, matched boxes ----------
    gidx_t = singles.tile([ML, 1], I32)
    nc.sync.dma_start(out=gidx_t, in_=ins["gidx"][:, :])
    ngidx_t = singles.tile([NL, 1], I32)
    nc.sync.dma_start(out=ngidx_t, in_=ins["ngidx"][:, :])

    pos16 = singles.tile([ML, RD], F16)
    nc.gpsimd.indirect_dma_start(
        out=pos16[:], out_offset=None, in_=ins["reg_n"][:, :],
        in_offset=bass.IndirectOffsetOnAxis(ap=gidx_t[:, 0:1], axis=0))
    neg16 = singles.tile([NL, RD], F16)
    nc.gpsimd.indirect_dma_start(
        out=neg16[:], out_offset=None, in_=ins["reg_n"][:, :],
        in_offset=bass.IndirectOffsetOnAxis(ap=ngidx_t[:, 0:1], axis=0))
    pos = singles.tile([ML, RD], F32)
    nc.vector.tensor_copy(out=pos, in_=pos16)
    neg = singles.tile([NL, RD], F32)
    nc.vector.tensor_copy(out=neg, in_=neg16)
    sbx = singles.tile([ML, 4], F32)
    nc.gpsimd.indirect_dma_start(
        out=sbx[:], out_offset=None, in_=ins["bbox_rows"][:, :],
        in_offset=bass.IndirectOffsetOnAxis(ap=gidx_t[:, 0:1], axis=0))

    # normalize pos / neg region rows
    n2p = singles.tile([ML, 1], F32)
    nc.scalar.activation(out=junkM, in_=pos, func=AF.Square,
                         accum_out=n2p[:])
    lnp = singles.tile([ML, 1], F32)
    nc.scalar.activation(out=lnp, in_=n2p, func=AF.Ln)
    nip = singles.tile([ML, 1], F32)
    nc.scalar.activation(out=nip, in_=lnp, func=AF.Exp, scale=-0.5)
    posn = singles.tile([ML, RD], F32)
    nc.vector.tensor_scalar(out=posn, in0=pos, scalar1=nip[:], scalar2=None,
                            op0=OP.mult)
    n2n = singles.tile([NL, 1], F32)
    junkN = junkR[0:NL, :]
    nc.scalar.activation(out=junkN, in_=neg, func=AF.Square,
                         accum_out=n2n[:])
    lnn = singles.tile([NL, 1], F32)
    nc.scalar.activation(out=lnn, in_=n2n, func=AF.Ln)
    nin = singles.tile([NL, 1], F32)
    nc.scalar.activation(out=nin, in_=lnn, func=AF.Exp, scale=-0.5)
    negn = singles.tile([NL, RD], F32)
    nc.vector.tensor_scalar(out=negn, in0=neg, scalar1=nin[:], scalar2=None,
                            op0=OP.mult)

    # partials tile: cols = [sp_sum, xtgt_sum, l1_sum, g2_sum, diag_sum, 0..]
    P5 = singles.tile([128, 8], F32)
    nc.vector.memset(P5, 0.0)

    # diag: rowwise dot(txtrep, posn); 1/TEMP pre-folded into txtrep
    nc.vector.tensor_tensor(out=junkM, in0=txtrep, in1=posn, op=OP.mult)
    nc.vector.tensor_reduce(out=P5[:, 4:5], in_=junkM,
                            axis=mybir.AxisListType.X, op=OP.add)

    # ---------------- loss_sim column block: S32 = txtn @ [posn|negn]^T ----
    arT0 = singles.tile([128, ML + NL], F32)
    arT1 = singles.tile([128, ML + NL], F32)
    for k, dst in ((0, arT0), (1, arT1)):
        ps_a = psum1.tile([128, ML], F32, tag="ps_one")
        nc.tensor.transpose(out=ps_a[:], in_=posn[:, k * 128:(k + 1) * 128],
                            identity=ident[:])
        nc.vector.tensor_copy(out=dst[:, 0:ML], in_=ps_a)
        ps_b = psum1.tile([128, NL], F32, tag="ps_one")
        nc.tensor.transpose(out=ps_b[:], in_=negn[:, k * 128:(k + 1) * 128],
                            identity=ident[0:NL, 0:NL])
        nc.vector.tensor_copy(out=dst[:, ML:ML + NL], in_=ps_b)
    ps_s = psum1.tile([B, ML + NL], F32, tag="ps_one")
    nc.tensor.matmul(out=ps_s[:], lhsT=txtT0_32[:], rhs=arT0[:], start=True,
                     stop=False)
    nc.tensor.matmul(out=ps_s[:], lhsT=txtT1_32[:], rhs=arT1[:], start=False,
                     stop=True)
    expS = singles.tile([B, ML + NL], F32)
    expsum = singles.tile([B, 1], F32)
    nc.scalar.activation(out=expS, in_=ps_s, func=AF.Exp, scale=1.0 / TEMP,
                         accum_out=expsum[:])

    # ---------------- cls loss partials ----------------
    clst = singles.tile([128, NT], F32)
    nc.sync.dma_start(out=clst, in_=ins["clsm"][:, :])
    # softplus(x) = relu(x) + ln(1 + exp(-|x|)) -- stable
    spa = singles.tile([128, NT], F32)
    nc.scalar.activation(out=spa, in_=clst, func=AF.Abs)
    spe = singles.tile([128, NT], F32)
    nc.scalar.activation(out=spe, in_=spa, func=AF.Exp, scale=-1.0)
    nc.vector.tensor_scalar(out=spe, in0=spe, scalar1=1.0, scalar2=None,
                            op0=OP.add)
    spl = singles.tile([128, NT], F32)
    nc.scalar.activation(out=spl, in_=spe, func=AF.Ln)
    spr = singles.tile([128, NT], F32)
    nc.vector.tensor_scalar(out=spr, in0=clst, scalar1=0.0, scalar2=None,
                            op0=OP.max)
    junkT = singles.tile([128, NT], F32, tag="junkT")
    nc.vector.tensor_tensor(out=junkT, in0=spl, in1=spr, op=OP.add)
    nc.vector.tensor_reduce(out=P5[:, 0:1], in_=junkT,
                            axis=mybir.AxisListType.X, op=OP.add)
    # scatter ones -> mask at matched query rows (dup-safe), then read back
    zeroT = singles.tile([128, NT], F32, tag="zeroT")
    nc.vector.memset(zeroT, 0.0)
    msk_dst = bass.AP(tensor=outs["mask_scratch"].tensor, offset=0,
                      ap=[[1, 128], [128, NT]])
    nc.sync.dma_start(out=msk_dst, in_=zeroT[:])
    onesML = singles.tile([ML, 1], F32)
    nc.vector.memset(onesML, 1.0)
    nc.gpsimd.indirect_dma_start(
        out=outs["mask_scratch"][:, :],
        out_offset=bass.IndirectOffsetOnAxis(ap=gidx_t[:, 0:1], axis=0),
        in_=onesML[:], in_offset=None)
    maskt = singles.tile([128, NT], F32)
    msk_src = bass.AP(tensor=outs["mask_scratch"].tensor, offset=0,
                      ap=[[1, 128], [128, NT]])
    nc.sync.dma_start(out=maskt, in_=msk_src)
    junkT2 = singles.tile([128, NT], F32, tag="junkT2")
    nc.vector.tensor_tensor(out=junkT2, in0=maskt, in1=clst, op=OP.mult)
    nc.vector.tensor_reduce(out=P5[:, 1:2], in_=junkT2,
                            axis=mybir.AxisListType.X, op=OP.add)

    # ---------------- matched-pair L1 and GIoU ----------------
    tl = singles.tile([ML, 4], F32)
    nc.sync.dma_start(out=tl, in_=ins["tgt_loc"][:, :])
    d4 = singles.tile([ML, 4], F32)
    nc.vector.tensor_sub(d4, sbx, tl)
    junk4 = singles.tile([ML, 4], F32, tag="junk4")
    nc.scalar.activation(out=junk4, in_=d4, func=AF.Abs,
                         accum_out=P5[:, 2:3])

    lt2 = singles.tile([ML, 2], F32)
    rb2 = singles.tile([ML, 2], F32)
    nc.vector.tensor_tensor(out=lt2, in0=sbx[:, 0:2], in1=tl[:, 0:2],
                            op=OP.max)
    nc.vector.tensor_tensor(out=rb2, in0=sbx[:, 2:4], in1=tl[:, 2:4],
                            op=OP.min)
    wh2 = singles.tile([ML, 2], F32)
    nc.vector.tensor_sub(wh2, rb2, lt2)
    whr = singles.tile([ML, 2], F32)
    nc.vector.tensor_scalar(out=whr, in0=wh2, scalar1=0.0, scalar2=None,
                            op0=OP.max)
    inter = singles.tile([ML, 1], F32)
    nc.vector.tensor_mul(inter, whr[:, 0:1], whr[:, 1:2])
    wa = singles.tile([ML, 1], F32)
    ha = singles.tile([ML, 1], F32)
    a1_ = singles.tile([ML, 1], F32)
    nc.vector.tensor_sub(wa, sbx[:, 2:3], sbx[:, 0:1])
    nc.vector.tensor_sub(ha, sbx[:, 3:4], sbx[:, 1:2])
    nc.vector.tensor_mul(a1_, wa, ha)
    wb_ = singles.tile([ML, 1], F32)
    hb_ = singles.tile([ML, 1], F32)
    a2_ = singles.tile([ML, 1], F32)
    nc.vector.tensor_sub(wb_, tl[:, 2:3], tl[:, 0:1])
    nc.vector.tensor_sub(hb_, tl[:, 3:4], tl[:, 1:2])
    nc.vector.tensor_mul(a2_, wb_, hb_)
    uni = singles.tile([ML, 1], F32)
    nc.vector.scalar_tensor_tensor(out=uni, in0=inter, scalar=-1.0, in1=a1_,
                                   op0=OP.mult, op1=OP.add)
    nc.vector.tensor_add(uni, uni, a2_)
    lte = singles.tile([ML, 2], F32)
    rbe = singles.tile([ML, 2], F32)
    nc.vector.tensor_tensor(out=lte, in0=sbx[:, 0:2], in1=tl[:, 0:2],
                            op=OP.min)
    nc.vector.tensor_tensor(out=rbe, in0=sbx[:, 2:4], in1=tl[:, 2:4],
                            op=OP.max)
    whe = singles.tile([ML, 2], F32)
    nc.vector.tensor_sub(whe, rbe, lte)
    enc = singles.tile([ML, 1], F32)
    nc.vector.tensor_mul(enc, whe[:, 0:1], whe[:, 1:2])
    lnu2 = singles.tile([ML, 1], F32)
    nc.scalar.activation(out=lnu2, in_=uni, func=AF.Ln)
    ru2 = singles.tile([ML, 1], F32)
    nc.scalar.activation(out=ru2, in_=lnu2, func=AF.Exp, scale=-1.0)
    lne2 = singles.tile([ML, 1], F32)
    nc.scalar.activation(out=lne2, in_=enc, func=AF.Ln)
    re2 = singles.tile([ML, 1], F32)
    nc.scalar.activation(out=re2, in_=lne2, func=AF.Exp, scale=-1.0)
    t1g = singles.tile([ML, 1], F32)
    t2g = singles.tile([ML, 1], F32)
    nc.vector.tensor_mul(t1g, inter, ru2)
    nc.vector.tensor_mul(t2g, uni, re2)
    junk1 = singles.tile([ML, 1], F32, tag="junk1")
    nc.vector.tensor_tensor(out=junk1, in0=t1g, in1=t2g, op=OP.add)
    nc.vector.tensor_reduce(out=P5[:, 3:4], in_=junk1,
                            axis=mybir.AxisListType.X, op=OP.add)

    # ---------------- reduce partials across partitions, write out -------
    ones128 = singles.tile([128, 1], F32)
    nc.vector.memset(ones128, 1.0)
    ps_l = psum1.tile([8, 1], F32, tag="ps_one")
    nc.tensor.matmul(out=ps_l[:], lhsT=P5[:], rhs=ones128[:], start=True,
                     stop=True)
    ls8 = singles.tile([8, 1], F32)
    nc.vector.tensor_copy(out=ls8, in_=ps_l)
    nc.sync.dma_start(out=outs["loss_out"][0:8], in_=ls8[:])
    nc.sync.dma_start(out=outs["loss_out"][8:8 + B], in_=expsum[:])


_NC_CACHE = None


def _get_program():
    global _NC_CACHE
    if _NC_CACHE is None:
        _NC_CACHE = build_program()
    return _NC_CACHE


def make_in_maps(inputs):
    """Shard + marshal FULL inputs into 8 per-core input maps."""
    rf = np.ascontiguousarray(inputs["region_features"], np.float32)
    bb = np.ascontiguousarray(inputs["bbox_pred"], np.float32)
    cp = np.ascontiguousarray(inputs["cls_pred"], np.float32)
    tb = np.ascontiguousarray(inputs["tgt_boxes"], np.float32)
    te = np.ascontiguousarray(inputs["text_embeddings"], np.float32)
    pi = np.ascontiguousarray(inputs["pred_idx"], np.int32)
    ni = np.ascontiguousarray(inputs["neg_idx"], np.int32)
    Wt = np.ascontiguousarray(inputs["Wt"], np.float32)
    bt = np.ascontiguousarray(inputs["bt"], np.float32)

    tgt_T = np.ascontiguousarray(tb.reshape(J, 4).T)          # [4, J]
    text_T = np.ascontiguousarray(te.T)                       # [TD, B]
    bt_row = bt.reshape(1, PD)

    in_maps = []
    for k in range(NCORES):
        gb = slice(k * BL, (k + 1) * BL)
        reg = rf[gb].reshape(QL, RD)
        reg_n = np.zeros((QP, RD), np.float16)
        reg_n[:QL] = reg.astype(np.float16)
        reg_t = np.ascontiguousarray(reg_n.T)
        bbox = bb[gb].reshape(QL, 4)
        bbox_rows = np.zeros((QP, 4), np.float32)
        bbox_rows[:QL] = bbox
        bboxm = np.ascontiguousarray(
            bbox_rows.reshape(NT, 128, 4).transpose(1, 0, 2).reshape(128, NT * 4))
        cls = np.full(QP, -50.0, np.float32)
        cls[:QL] = cp[gb].reshape(QL)
        clsm = np.ascontiguousarray(cls.reshape(NT, 128).T)
        tgt_loc = np.ascontiguousarray(tb[gb].reshape(ML, 4))
        text_rep = np.repeat(te[gb], T, axis=0)               # [ML, TD]
        text_rep_T = np.ascontiguousarray(text_rep.T)
        loc_off = (np.arange(BL, dtype=np.int32) * Q)[:, None]
        gidx = (pi[gb] + loc_off).reshape(ML, 1).astype(np.int32)
        ngidx = (ni[gb] + loc_off).reshape(NL, 1).astype(np.int32)
        in_maps.append(dict(
            reg_n=reg_n, reg_t=reg_t, bboxm=bboxm, bbox_rows=bbox_rows,
            clsm=clsm, tgt_T=tgt_T, tgt_loc=tgt_loc, text_T=text_T,
            text_rep_T=text_rep_T, Wt=Wt, bt_row=bt_row, gidx=gidx,
            ngidx=ngidx))
    return in_maps


def combine(results):
    """Combine per-core outputs into the full flat reference output."""
    C = np.empty((B, Q, J), np.float32)
    sp = xt = l1s = g2s = dg = 0.0
    expsum = np.zeros(B, np.float64)
    for k, r in enumerate(results):
        C[k * BL:(k + 1) * BL] = \
            r["C_out"][:QL].astype(np.float32).reshape(BL, Q, J)
        lo = r["loss_out"].astype(np.float64)
        sp += lo[0]
        xt += lo[1]
        l1s += lo[2]
        g2s += lo[3]
        dg += lo[4]
        expsum += lo[8:8 + B]
    loss_cls = 2.0 * (sp - xt) / (B * Q)
    loss_l1 = 5.0 * l1s / (B * T * 4)
    giou_mean = (g2s - B * T) / (B * T)
    loss_giou = 2.0 * (1.0 - giou_mean)
    loss_sim = np.mean(np.log(expsum)) - dg / (B * T)
    losses = np.array([loss_cls, loss_l1, loss_giou, loss_sim], np.float32)
    return np.concatenate([C.reshape(-1), losses])


def run(inputs, trace=False, **kw):
    nc = _get_program()
    in_maps = make_in_maps(inputs)
    try:
        res = run_bass_kernel_spmd(nc, in_maps, core_ids=list(range(NCORES)),
                                   trace=trace, **kw)
    except ModuleNotFoundError:
        res = run_bass_kernel_spmd(nc, in_maps, core_ids=list(range(NCORES)),
                                   trace=False, **kw)
    return combine(res.results), res


def kernel(**inputs) -> np.ndarray:
    out, _ = run(inputs)
    return out


if __name__ == "__main__":
    import reference
    inputs = {k: np.asarray(v) for k, v in reference.setup_inputs().items()}
    out = kernel(**inputs)
    exp = np.asarray(reference.reference(**inputs))
    err = np.abs(out - exp)
    scale = np.abs(exp).max()
    print("max abs err:", err.max(), " scale:", scale,
          " rel:", err.max() / scale)
